# revision 20
# baseline (speedup 1.0000x reference)
"""Trainium2 Bass kernel for DigitConvolutionalModel.

Math: the 3x3 valid conv on the 28x28 image is a linear map, so it folds into
the first Linear layer:
    out = relu(x @ W_eff + b1) @ w2.T + b2
where W_eff[784, 128] = C @ w1.T and C[784, 676] is the conv-as-matrix built
from conv_w.  W_eff is built on the host (O(1) w.r.t. batch); the device does
the two batch matmuls.  +b2 is added on the host after the gather.

Distribution: pure data parallel — batch dim of x sharded across 8 NeuronCores,
weights replicated.  Each core computes out.T [10, 8192] fp16; the host
reassembles [65536, 10] fp32.

dtypes: x ships as fp8 e3m4 (4 mantissa bits; measured max-err 1.3e-2 of
output scale vs the 2e-2 gate) — halves HBM traffic vs fp16 and makes the
kernel PE-bound.  W_eff stays fp16 (mixed fp8xfp16 matmul runs at the full
1 col/cycle rate).  PSUM accumulates fp32; h = relu(psum+b1) on DVE as fp16.

PE shape discipline (v3): K is zero-padded 784 -> 896 = 7x128 and w2 is
zero-padded [10,128] -> [128,128], so EVERY matmul in the kernel has a
[128,128] stationary operand — no row_grp/col_grp PE-array reconfig bubbles
between the mm1 k-tiles, the old K=16 remainder, and mm2 (measured ~100 ns
per switch, ~300 ns/tile).

DMA discipline (v3): all weights are host-prepacked into ONE fp16 tensor
wpack [128, 1024] (contiguous 2 KB/partition descriptors) — the v2 rearrange
pattern emitted 768x256B descriptors that crawled at 26 GB/s and stalled the
PE ~5 us waiting on w2t/b1.  x rides the sync ring alone (measured 338 GB/s);
wpack + b1 ride the scalar ring.  The first two x DMAs are sub-tile slices so
the first matmul can start ~4 us in; ~40 128-col warmup matmuls keep the PE
HAM clock gate open across the fill (v2's 30 fell 240 ns short of the 3.4 us
activity window and the first ~19 real matmuls ran at 1.2 GHz).
"""

import numpy as np
import ml_dtypes

import concourse.bass as bass  # noqa: F401  (bass registers mybir lowerings)
import concourse.mybir as mybir
import concourse.tile as tile
from concourse import bacc
from concourse.bass_utils import run_bass_kernel_spmd

N_CORES = 8
B = 65536
B_SH = B // N_CORES  # 8192 rows per core
D = 784              # 28*28 input features
H = 128              # hidden
OUT = 10
KT = 128             # contraction tile = full partition dim
NK = 7               # K-tiles (784 zero-padded to 896 = 7*128)
DP = NK * KT         # 896 padded features
NB = 512             # batch columns per tile (= one fp32 PSUM bank)
NT = B_SH // NB      # 16 batch tiles
W2COL = DP           # wpack column where padded w2 starts
N_WARM = 50          # warmup matmuls covering the DMA fill (~4.6 us)

_CACHE = {}


def _build_nc():
    f32 = mybir.dt.float32
    f16 = mybir.dt.float16
    f8 = mybir.dt.float8e3
    nc = bacc.Bacc("TRN2", target_bir_lowering=False, debug=False,
                   num_devices=N_CORES)
    # main x, partition-major: [p, t, k, c] with feature f = k*128 + p
    xtp = nc.dram_tensor("xtp", [KT, NT, NK, NB], f8,
                         kind="ExternalInput").ap()
    # all fp16 weights in one clean-descriptor pack:
    # cols [k*128, k*128+128) = W_eff k-tile k (row k*128+p -> col m);
    # cols [896, 1024) = w2.T zero-padded to [128, 128].
    wpack = nc.dram_tensor("wpack", [KT, DP + KT], f16,
                           kind="ExternalInput").ap()
    b1c = nc.dram_tensor("b1c", [H, 1], f32, kind="ExternalInput").ap()
    out = nc.dram_tensor("out", [OUT, B_SH], f16, kind="ExternalOutput").ap()

    with tile.TileContext(nc) as tc:
        with (
            tc.tile_pool(name="wpool", bufs=1) as wpool,
            tc.tile_pool(name="xpool", bufs=1) as xpool,
            tc.tile_pool(name="hpool", bufs=4) as hpool,
            tc.tile_pool(name="opool", bufs=1) as opool,
            tc.tile_pool(name="ps1", bufs=4, space="PSUM") as ps1pool,
            tc.tile_pool(name="ps2", bufs=3, space="PSUM") as ps2pool,
            tc.tile_pool(name="psw", bufs=1, space="PSUM") as pswpool,
        ):
            # The whole x shard lives in SBUF (7.3 MB) — no recycling, the
            # stream never waits on compute.
            x_all = xpool.tile([KT, NT, NK, NB], f8)
            w_sb = wpool.tile([KT, DP + KT], f16)
            b1_sb = wpool.tile([H, 1], f32)

            # sync ring: x only (the sync/SP HWDGE queue drains ~340 GB/s;
            # the scalar/ACT one only ~140 GB/s — measured).  Tile 0 ships as
            # ONE transfer: the DMA completion semaphore lags the last byte
            # by ~2.5 us with ~1 us jitter, so every extra early completion
            # event is a fresh chance to stall the PE and re-cool the HAM
            # clock gate (slicing tile 0 finer measured faster on lucky runs
            # but 2-3 us slower on unlucky ones).
            nc.sync.dma_start(x_all[:, 0:1, :, :], xtp[:, 0:1, :, :])
            for a, b in ((1, 2), (2, 3), (4, 6), (6, 8), (8, 10),
                         (10, 12), (12, 14), (14, 16)):
                nc.sync.dma_start(x_all[:, a:b, :, :], xtp[:, a:b, :, :])
            # scalar ring: the small weight pack (k0-3 first), b1, and tile 3
            # (the scalar queue is idle after ~11 us; offloading one early
            # tile lets t1/t2 land sooner on the sync queue)
            nc.scalar.dma_start(w_sb[:, 0:4 * KT], wpack[:, 0:4 * KT])
            nc.scalar.dma_start(b1_sb[:], b1c[:])
            nc.scalar.dma_start(w_sb[:, 4 * KT:], wpack[:, 4 * KT:])
            nc.scalar.dma_start(x_all[:, 3:4, :, :], xtp[:, 3:4, :, :])

            # PE pre-warm: 128-col matmuls on a zeroed tile keep the HAM
            # activity monitor at full clock while the first x lands.
            warm_x = wpool.tile([KT, KT], f8)
            nc.vector.memset(warm_x[:], 0.0)
            warm_ps = pswpool.tile([H, KT], f32)
            for _ in range(N_WARM):
                nc.tensor.matmul(warm_ps[:], lhsT=warm_x[:],
                                 rhs=warm_x[:], start=True, stop=True)

            o_all = opool.tile([OUT, NT, NB], f16)

            def epilogue(t, ps1, split=1):
                # h = relu(ps1 + b1), fused on DVE, emitted as fp16.  The
                # last tile runs with split=2: the halves' relu/copy go to
                # the vector and scalar(ACT) engines IN PARALLEL so the final
                # store starts ~1 us earlier (a serial DVE split measured
                # slower than no split at all).
                ncol = NB // split
                for s in range(split):
                    cs = slice(s * ncol, (s + 1) * ncol)
                    h_sb = hpool.tile([H, ncol], f16)
                    if s == 0:
                        nc.vector.tensor_scalar(
                            h_sb[:], ps1[:, cs], b1_sb[:], 0.0,
                            mybir.AluOpType.add, mybir.AluOpType.max)
                    else:
                        # second half on the ACT engine (gpsimd cannot read
                        # PSUM): h = relu(ps1 + b1)
                        nc.scalar.activation(
                            h_sb[:], ps1[:, cs],
                            mybir.ActivationFunctionType.Relu, bias=b1_sb[:])
                    # out.T[10, ncol] = w2 @ h.T (w2 zero-padded to [128,128]
                    # so the stationary shape never changes; only rows 0:10
                    # are read out)
                    ps2 = ps2pool.tile([H, ncol], f32)
                    nc.tensor.matmul(ps2[:], lhsT=w_sb[:, W2COL:W2COL + KT],
                                     rhs=h_sb[:], start=True, stop=True)
                    if s == 0:
                        nc.vector.tensor_copy(o_all[:, t, cs], ps2[0:OUT, :])
                    else:
                        nc.scalar.activation(
                            o_all[:, t, cs], ps2[0:OUT, :],
                            mybir.ActivationFunctionType.Copy)

            pending = None  # software pipeline: tile t's epilogue is emitted
                            # after tile t+1's mm1 block so PE never waits on
                            # the DVE relu chain
            for t in range(NT):
                # h.T[128, NB] = W_eff.T @ x.T, accumulated over 7 K-tiles.
                ps1 = ps1pool.tile([H, NB], f32)
                for k in range(NK):
                    nc.tensor.matmul(
                        ps1[:],
                        lhsT=w_sb[:, k * KT:(k + 1) * KT],
                        rhs=x_all[:, t, k, :],
                        start=(k == 0),
                        stop=(k == NK - 1),
                    )
                if pending is not None:
                    epilogue(*pending)
                pending = (t, ps1)
            epilogue(*pending, split=2)

            # out stores in 2-tile chunks on the (now idle) sync ring; each
            # chunk only waits on its own slice writes, so they pipeline with
            # the epilogue tail.
            for c in range(8):
                nc.sync.dma_start(out[:, c * 2 * NB:(c + 1) * 2 * NB],
                                  o_all[:, c * 2:(c + 1) * 2, :])

    nc.compile()
    return nc


def _get_nc():
    if "nc" not in _CACHE:
        _CACHE["nc"] = _build_nc()
    return _CACHE["nc"]


def _fold_weights(conv_w: np.ndarray, w1: np.ndarray) -> np.ndarray:
    """W_eff[784, 128]: h_pre = x @ W_eff  ==  conv(x) @ w1.T  (float64 accum)."""
    w1k = w1.reshape(H, 26, 26).transpose(1, 2, 0).astype(np.float64)  # [i,j,k]
    cw = conv_w.astype(np.float64)
    W = np.zeros((28, 28, H), np.float64)
    for di in range(3):
        for dj in range(3):
            W[di:di + 26, dj:dj + 26, :] += cw[di, dj] * w1k
    return W.reshape(D, H).astype(np.float32)


def make_in_maps(x, conv_w, w1, b1, w2, b2):
    x = np.asarray(x, np.float32)
    weff = _fold_weights(np.asarray(conv_w, np.float32),
                         np.asarray(w1, np.float32))
    # wpack[p, k*128+m] = weff_pad[k*128+p, m]; wpack[p, 896+j] = w2.T padded
    wpack = np.zeros((KT, DP + KT), np.float16)
    weff_pad = np.zeros((DP, H), np.float32)
    weff_pad[:D] = weff
    wpack[:, :DP] = (weff_pad.reshape(NK, KT, H).transpose(1, 0, 2)
                     .reshape(KT, DP).astype(np.float16))
    wpack[:, DP:DP + OUT] = np.asarray(w2, np.float32).T.astype(np.float16)
    b1c = np.ascontiguousarray(np.asarray(b1, np.float32).reshape(H, 1))
    in_maps = []
    for i in range(N_CORES):
        xs = np.zeros((B_SH, DP), ml_dtypes.float8_e3m4)
        xs[:, :D] = x[i * B_SH:(i + 1) * B_SH].astype(ml_dtypes.float8_e3m4)
        # main: [t*NB+c, k*KT+p] -> [p, t, k, c]
        xtp = xs.reshape(NT, NB, NK, KT).transpose(3, 0, 2, 1)
        in_maps.append({"xtp": np.ascontiguousarray(xtp),
                        "wpack": wpack, "b1c": b1c})
    return in_maps


def kernel(x, conv_w, w1, b1, w2, b2):
    nc = _get_nc()
    in_maps = make_in_maps(x, conv_w, w1, b1, w2, b2)
    res = run_bass_kernel_spmd(nc, in_maps, list(range(N_CORES)))
    out = np.concatenate([res.results[i]["out"] for i in range(N_CORES)], axis=1)
    # [10, 65536] fp16 -> [65536, 10] fp32, + b2 (host-side fold)
    return np.ascontiguousarray(out.T.astype(np.float32)
                                + np.asarray(b2, np.float32))


# revision 21
# speedup vs baseline: 1.0730x; 1.0730x over previous
"""Trainium2 Bass kernel for DigitConvolutionalModel.

Math: the 3x3 valid conv on the 28x28 image is a linear map, so it folds into
the first Linear layer:
    out = relu(x @ W_eff + b1) @ w2.T + b2
where W_eff[784, 128] = C @ w1.T and C[784, 676] is the conv-as-matrix built
from conv_w.  W_eff is built on the host (O(1) w.r.t. batch); the device does
the two batch matmuls.  +b2 is added on the host after the gather.

Distribution: pure data parallel — batch dim of x sharded across 8 NeuronCores,
weights replicated.  Each core computes out.T [10, 8192] fp16; the host
reassembles [65536, 10] fp32.

dtypes: x ships as fp8 e3m4 (4 mantissa bits; measured max-err 1.3e-2 of
output scale vs the 2e-2 gate) — halves HBM traffic vs fp16 and makes the
kernel PE-bound.  W_eff stays fp16 (mixed fp8xfp16 matmul runs at the full
1 col/cycle rate).  PSUM accumulates fp32; h = relu(psum+b1) on DVE as fp16.

PE shape discipline (v3): K is zero-padded 784 -> 896 = 7x128 and w2 is
zero-padded [10,128] -> [128,128], so EVERY matmul in the kernel has a
[128,128] stationary operand — no row_grp/col_grp PE-array reconfig bubbles
between the mm1 k-tiles, the old K=16 remainder, and mm2 (measured ~100 ns
per switch, ~300 ns/tile).

DMA discipline (v3): all weights are host-prepacked into ONE fp16 tensor
wpack [128, 1024] (contiguous 2 KB/partition descriptors) — the v2 rearrange
pattern emitted 768x256B descriptors that crawled at 26 GB/s and stalled the
PE ~5 us waiting on w2t/b1.  x rides the sync ring alone (measured 338 GB/s);
wpack + b1 ride the scalar ring.  The first two x DMAs are sub-tile slices so
the first matmul can start ~4 us in; ~40 128-col warmup matmuls keep the PE
HAM clock gate open across the fill (v2's 30 fell 240 ns short of the 3.4 us
activity window and the first ~19 real matmuls ran at 1.2 GHz).
"""

import numpy as np
import ml_dtypes

import concourse.bass as bass  # noqa: F401  (bass registers mybir lowerings)
import concourse.mybir as mybir
import concourse.tile as tile
from concourse import bacc
from concourse.bass_utils import run_bass_kernel_spmd

N_CORES = 8
B = 65536
B_SH = B // N_CORES  # 8192 rows per core
D = 784              # 28*28 input features
H = 128              # hidden
OUT = 10
KT = 128             # contraction tile = full partition dim
NK = 7               # K-tiles (784 zero-padded to 896 = 7*128)
DP = NK * KT         # 896 padded features
NB = 512             # batch columns per tile (= one fp32 PSUM bank)
NT = B_SH // NB      # 16 batch tiles
W2COL = DP           # wpack column where padded w2 starts
N_WARM = 50          # warmup matmuls covering the DMA fill (~4.6 us)

_CACHE = {}


def _build_nc():
    f32 = mybir.dt.float32
    f16 = mybir.dt.float16
    f8 = mybir.dt.float8e3
    nc = bacc.Bacc("TRN2", target_bir_lowering=False, debug=False,
                   num_devices=N_CORES)
    # main x, partition-major: [p, t, k, c] with feature f = k*128 + p
    xtp = nc.dram_tensor("xtp", [KT, NT, NK, NB], f8,
                         kind="ExternalInput").ap()
    # all fp16 weights in one clean-descriptor pack:
    # cols [k*128, k*128+128) = W_eff k-tile k (row k*128+p -> col m);
    # cols [896, 1024) = w2.T zero-padded to [128, 128].
    wpack = nc.dram_tensor("wpack", [KT, DP + KT], f16,
                           kind="ExternalInput").ap()
    b1c = nc.dram_tensor("b1c", [H, 1], f32, kind="ExternalInput").ap()
    out = nc.dram_tensor("out", [OUT, B_SH], f16, kind="ExternalOutput").ap()

    with tile.TileContext(nc) as tc:
        with (
            tc.tile_pool(name="wpool", bufs=1) as wpool,
            tc.tile_pool(name="xpool", bufs=1) as xpool,
            tc.tile_pool(name="hpool", bufs=4) as hpool,
            tc.tile_pool(name="opool", bufs=1) as opool,
            tc.tile_pool(name="ps1", bufs=4, space="PSUM") as ps1pool,
            tc.tile_pool(name="ps2", bufs=3, space="PSUM") as ps2pool,
            tc.tile_pool(name="psw", bufs=1, space="PSUM") as pswpool,
        ):
            # The whole x shard lives in SBUF (7.3 MB) — no recycling, the
            # stream never waits on compute.
            x_all = xpool.tile([KT, NT, NK, NB], f8)
            w_sb = wpool.tile([KT, DP + KT], f16)
            b1_sb = wpool.tile([H, 1], f32)

            # The DMA completion semaphore lags the last byte by ~2.5 us with
            # ~1 us jitter, and an early-window PE stall re-cools the HAM
            # clock gate (turning one late sem into a 3-4 us cascade), so
            # the start is built on as FEW completion events as possible:
            # tile 0 and the weight pack each ship as ONE transfer, both on
            # the fast sync/SP queue (~340 GB/s; the scalar/ACT queue only
            # drains ~140 GB/s — measured).  t3 rides the otherwise-idle
            # scalar queue so t1/t2 land sooner on sync.
            nc.sync.dma_start(x_all[:, 0:1, :, :], xtp[:, 0:1, :, :])
            nc.sync.dma_start(w_sb[:], wpack[:])
            for a, b in ((1, 2), (2, 3), (4, 6), (6, 8), (8, 10),
                         (10, 12), (12, 14), (14, 16)):
                nc.sync.dma_start(x_all[:, a:b, :, :], xtp[:, a:b, :, :])
            nc.scalar.dma_start(b1_sb[:], b1c[:])
            nc.scalar.dma_start(x_all[:, 3:4, :, :], xtp[:, 3:4, :, :])

            # PE pre-warm: 128-col matmuls on a zeroed tile keep the HAM
            # activity monitor at full clock while the first x lands.
            warm_x = wpool.tile([KT, KT], f8)
            nc.vector.memset(warm_x[:], 0.0)
            warm_ps = pswpool.tile([H, KT], f32)
            for _ in range(N_WARM):
                nc.tensor.matmul(warm_ps[:], lhsT=warm_x[:],
                                 rhs=warm_x[:], start=True, stop=True)

            o_all = opool.tile([OUT, NT, NB], f16)

            def epilogue(t, ps1, split=1):
                # h = relu(ps1 + b1), fused on DVE, emitted as fp16.  The
                # last tile runs with split=2: the halves' relu/copy go to
                # the vector and scalar(ACT) engines IN PARALLEL so the final
                # store starts ~1 us earlier (a serial DVE split measured
                # slower than no split at all).
                ncol = NB // split
                for s in range(split):
                    cs = slice(s * ncol, (s + 1) * ncol)
                    h_sb = hpool.tile([H, ncol], f16)
                    if s == 0:
                        nc.vector.tensor_scalar(
                            h_sb[:], ps1[:, cs], b1_sb[:], 0.0,
                            mybir.AluOpType.add, mybir.AluOpType.max)
                    else:
                        # second half on the ACT engine (gpsimd cannot read
                        # PSUM): h = relu(ps1 + b1)
                        nc.scalar.activation(
                            h_sb[:], ps1[:, cs],
                            mybir.ActivationFunctionType.Relu, bias=b1_sb[:])
                    # out.T[10, ncol] = w2 @ h.T (w2 zero-padded to [128,128]
                    # so the stationary shape never changes; only rows 0:10
                    # are read out)
                    ps2 = ps2pool.tile([H, ncol], f32)
                    nc.tensor.matmul(ps2[:], lhsT=w_sb[:, W2COL:W2COL + KT],
                                     rhs=h_sb[:], start=True, stop=True)
                    if s == 0:
                        nc.vector.tensor_copy(o_all[:, t, cs], ps2[0:OUT, :])
                    else:
                        nc.scalar.activation(
                            o_all[:, t, cs], ps2[0:OUT, :],
                            mybir.ActivationFunctionType.Copy)

            pending = None  # software pipeline: tile t's epilogue is emitted
                            # after tile t+1's mm1 block so PE never waits on
                            # the DVE relu chain
            for t in range(NT):
                # h.T[128, NB] = W_eff.T @ x.T, accumulated over 7 K-tiles.
                ps1 = ps1pool.tile([H, NB], f32)
                for k in range(NK):
                    nc.tensor.matmul(
                        ps1[:],
                        lhsT=w_sb[:, k * KT:(k + 1) * KT],
                        rhs=x_all[:, t, k, :],
                        start=(k == 0),
                        stop=(k == NK - 1),
                    )
                if pending is not None:
                    epilogue(*pending)
                pending = (t, ps1)
            epilogue(*pending, split=2)

            # out stores in 2-tile chunks on the (now idle) sync ring; each
            # chunk only waits on its own slice writes, so they pipeline with
            # the epilogue tail.
            for c in range(8):
                nc.sync.dma_start(out[:, c * 2 * NB:(c + 1) * 2 * NB],
                                  o_all[:, c * 2:(c + 1) * 2, :])

    nc.compile()
    return nc


def _get_nc():
    if "nc" not in _CACHE:
        _CACHE["nc"] = _build_nc()
    return _CACHE["nc"]


def _fold_weights(conv_w: np.ndarray, w1: np.ndarray) -> np.ndarray:
    """W_eff[784, 128]: h_pre = x @ W_eff  ==  conv(x) @ w1.T  (float64 accum)."""
    w1k = w1.reshape(H, 26, 26).transpose(1, 2, 0).astype(np.float64)  # [i,j,k]
    cw = conv_w.astype(np.float64)
    W = np.zeros((28, 28, H), np.float64)
    for di in range(3):
        for dj in range(3):
            W[di:di + 26, dj:dj + 26, :] += cw[di, dj] * w1k
    return W.reshape(D, H).astype(np.float32)


def make_in_maps(x, conv_w, w1, b1, w2, b2):
    x = np.asarray(x, np.float32)
    weff = _fold_weights(np.asarray(conv_w, np.float32),
                         np.asarray(w1, np.float32))
    # wpack[p, k*128+m] = weff_pad[k*128+p, m]; wpack[p, 896+j] = w2.T padded
    wpack = np.zeros((KT, DP + KT), np.float16)
    weff_pad = np.zeros((DP, H), np.float32)
    weff_pad[:D] = weff
    wpack[:, :DP] = (weff_pad.reshape(NK, KT, H).transpose(1, 0, 2)
                     .reshape(KT, DP).astype(np.float16))
    wpack[:, DP:DP + OUT] = np.asarray(w2, np.float32).T.astype(np.float16)
    b1c = np.ascontiguousarray(np.asarray(b1, np.float32).reshape(H, 1))
    in_maps = []
    for i in range(N_CORES):
        xs = np.zeros((B_SH, DP), ml_dtypes.float8_e3m4)
        xs[:, :D] = x[i * B_SH:(i + 1) * B_SH].astype(ml_dtypes.float8_e3m4)
        # main: [t*NB+c, k*KT+p] -> [p, t, k, c]
        xtp = xs.reshape(NT, NB, NK, KT).transpose(3, 0, 2, 1)
        in_maps.append({"xtp": np.ascontiguousarray(xtp),
                        "wpack": wpack, "b1c": b1c})
    return in_maps


def kernel(x, conv_w, w1, b1, w2, b2):
    nc = _get_nc()
    in_maps = make_in_maps(x, conv_w, w1, b1, w2, b2)
    res = run_bass_kernel_spmd(nc, in_maps, list(range(N_CORES)))
    out = np.concatenate([res.results[i]["out"] for i in range(N_CORES)], axis=1)
    # [10, 65536] fp16 -> [65536, 10] fp32, + b2 (host-side fold)
    return np.ascontiguousarray(out.T.astype(np.float32)
                                + np.asarray(b2, np.float32))


# revision 22
# speedup vs baseline: 1.0838x; 1.0100x over previous
"""Trainium2 Bass kernel for DigitConvolutionalModel.

Math: the 3x3 valid conv on the 28x28 image is a linear map, so it folds into
the first Linear layer:
    out = relu(x @ W_eff + b1) @ w2.T + b2
where W_eff[784, 128] = C @ w1.T and C[784, 676] is the conv-as-matrix built
from conv_w.  W_eff is built on the host (O(1) w.r.t. batch); the device does
the two batch matmuls.  +b2 is added on the host after the gather.

Distribution: pure data parallel — batch dim of x sharded across 8 NeuronCores,
weights replicated.  Each core computes out.T [10, 8192] fp16; the host
reassembles [65536, 10] fp32.

dtypes: x ships as fp8 e3m4 (4 mantissa bits; measured max-err 1.3e-2 of
output scale vs the 2e-2 gate) — halves HBM traffic vs fp16 and makes the
kernel PE-bound.  W_eff stays fp16 (mixed fp8xfp16 matmul runs at the full
1 col/cycle rate).  PSUM accumulates fp32; h = relu(psum+b1) on DVE as fp16.

PE shape discipline (v3): K is zero-padded 784 -> 896 = 7x128 and w2 is
zero-padded [10,128] -> [128,128], so EVERY matmul in the kernel has a
[128,128] stationary operand — no row_grp/col_grp PE-array reconfig bubbles
between the mm1 k-tiles, the old K=16 remainder, and mm2 (measured ~100 ns
per switch, ~300 ns/tile).

DMA discipline (v3): all weights are host-prepacked into ONE fp16 tensor
wpack [128, 1024] (contiguous 2 KB/partition descriptors) — the v2 rearrange
pattern emitted 768x256B descriptors that crawled at 26 GB/s and stalled the
PE ~5 us waiting on w2t/b1.  x rides the sync ring alone (measured 338 GB/s);
wpack + b1 ride the scalar ring.  The first two x DMAs are sub-tile slices so
the first matmul can start ~4 us in; ~40 128-col warmup matmuls keep the PE
HAM clock gate open across the fill (v2's 30 fell 240 ns short of the 3.4 us
activity window and the first ~19 real matmuls ran at 1.2 GHz).
"""

import numpy as np
import ml_dtypes

import concourse.bass as bass  # noqa: F401  (bass registers mybir lowerings)
import concourse.mybir as mybir
import concourse.tile as tile
from concourse import bacc
from concourse.bass_utils import run_bass_kernel_spmd

N_CORES = 8
B = 65536
B_SH = B // N_CORES  # 8192 rows per core
D = 784              # 28*28 input features
H = 128              # hidden
OUT = 10
KT = 128             # contraction tile = full partition dim
NK = 7               # K-tiles (784 zero-padded to 896 = 7*128)
DP = NK * KT         # 896 padded features
NB = 512             # batch columns per tile (= one fp32 PSUM bank)
NT = B_SH // NB      # 16 batch tiles
W2COL = DP           # wpack column where padded w2 starts
N_WARM = 50          # warmup matmuls covering the DMA fill (~4.6 us)

_CACHE = {}


def _build_nc():
    f32 = mybir.dt.float32
    f16 = mybir.dt.float16
    f8 = mybir.dt.float8e3
    nc = bacc.Bacc("TRN2", target_bir_lowering=False, debug=False,
                   num_devices=N_CORES)
    # main x, partition-major: [p, t, k, c] with feature f = k*128 + p
    xtp = nc.dram_tensor("xtp", [KT, NT, NK, NB], f8,
                         kind="ExternalInput").ap()
    # all fp16 weights in one clean-descriptor pack:
    # cols [k*128, k*128+128) = W_eff k-tile k (row k*128+p -> col m);
    # cols [896, 1024) = w2.T zero-padded to [128, 128].
    wpack = nc.dram_tensor("wpack", [KT, DP + KT], f16,
                           kind="ExternalInput").ap()
    b1c = nc.dram_tensor("b1c", [H, 1], f32, kind="ExternalInput").ap()
    out = nc.dram_tensor("out", [OUT, B_SH], f16, kind="ExternalOutput").ap()

    with tile.TileContext(nc) as tc:
        with (
            tc.tile_pool(name="wpool", bufs=1) as wpool,
            tc.tile_pool(name="xpool", bufs=1) as xpool,
            tc.tile_pool(name="hpool", bufs=4) as hpool,
            tc.tile_pool(name="opool", bufs=1) as opool,
            tc.tile_pool(name="ps1", bufs=4, space="PSUM") as ps1pool,
            tc.tile_pool(name="ps2", bufs=3, space="PSUM") as ps2pool,
            tc.tile_pool(name="psw", bufs=1, space="PSUM") as pswpool,
        ):
            # The whole x shard lives in SBUF (7.3 MB) — no recycling, the
            # stream never waits on compute.
            x_all = xpool.tile([KT, NT, NK, NB], f8)
            w_sb = wpool.tile([KT, DP + KT], f16)
            b1_sb = wpool.tile([H, 1], f32)

            # The DMA completion semaphore lags the last byte by ~2.5 us with
            # ~1 us jitter, and an early-window PE stall re-cools the HAM
            # clock gate (turning one late sem into a 3-4 us cascade), so
            # the start is built on as FEW completion events as possible:
            # tile 0 and the weight pack each ship as ONE transfer, both on
            # the fast sync/SP queue (~340 GB/s; the scalar/ACT queue only
            # drains ~140 GB/s — measured).  t3 rides the otherwise-idle
            # scalar queue so t1/t2 land sooner on sync.
            nc.sync.dma_start(x_all[:, 0:1, :, :], xtp[:, 0:1, :, :])
            for a, b in ((1, 2), (2, 3), (4, 6), (6, 8), (8, 10),
                         (10, 12), (12, 14), (14, 16)):
                nc.sync.dma_start(x_all[:, a:b, :, :], xtp[:, a:b, :, :])
            nc.scalar.dma_start(w_sb[:], wpack[:])
            nc.scalar.dma_start(b1_sb[:], b1c[:])
            nc.scalar.dma_start(x_all[:, 3:4, :, :], xtp[:, 3:4, :, :])

            # PE pre-warm: 128-col matmuls on a zeroed tile keep the HAM
            # activity monitor at full clock while the first x lands.
            warm_x = wpool.tile([KT, KT], f8)
            nc.vector.memset(warm_x[:], 0.0)
            warm_ps = pswpool.tile([H, KT], f32)
            for _ in range(N_WARM):
                nc.tensor.matmul(warm_ps[:], lhsT=warm_x[:],
                                 rhs=warm_x[:], start=True, stop=True)

            o_all = opool.tile([OUT, NT, NB], f16)

            def epilogue(t, ps1, split=1):
                # h = relu(ps1 + b1), fused on DVE, emitted as fp16.  The
                # last tile runs with split=2: the halves' relu/copy go to
                # the vector and scalar(ACT) engines IN PARALLEL so the final
                # store starts ~1 us earlier (a serial DVE split measured
                # slower than no split at all).
                ncol = NB // split
                for s in range(split):
                    cs = slice(s * ncol, (s + 1) * ncol)
                    h_sb = hpool.tile([H, ncol], f16)
                    if s == 0:
                        nc.vector.tensor_scalar(
                            h_sb[:], ps1[:, cs], b1_sb[:], 0.0,
                            mybir.AluOpType.add, mybir.AluOpType.max)
                    else:
                        # second half on the ACT engine (gpsimd cannot read
                        # PSUM): h = relu(ps1 + b1)
                        nc.scalar.activation(
                            h_sb[:], ps1[:, cs],
                            mybir.ActivationFunctionType.Relu, bias=b1_sb[:])
                    # out.T[10, ncol] = w2 @ h.T (w2 zero-padded to [128,128]
                    # so the stationary shape never changes; only rows 0:10
                    # are read out)
                    ps2 = ps2pool.tile([H, ncol], f32)
                    nc.tensor.matmul(ps2[:], lhsT=w_sb[:, W2COL:W2COL + KT],
                                     rhs=h_sb[:], start=True, stop=True)
                    if s == 0:
                        nc.vector.tensor_copy(o_all[:, t, cs], ps2[0:OUT, :])
                    else:
                        nc.scalar.activation(
                            o_all[:, t, cs], ps2[0:OUT, :],
                            mybir.ActivationFunctionType.Copy)

            pending = None  # software pipeline: tile t's epilogue is emitted
                            # after tile t+1's mm1 block so PE never waits on
                            # the DVE relu chain
            for t in range(NT):
                # h.T[128, NB] = W_eff.T @ x.T, accumulated over 7 K-tiles.
                ps1 = ps1pool.tile([H, NB], f32)
                for k in range(NK):
                    nc.tensor.matmul(
                        ps1[:],
                        lhsT=w_sb[:, k * KT:(k + 1) * KT],
                        rhs=x_all[:, t, k, :],
                        start=(k == 0),
                        stop=(k == NK - 1),
                    )
                if pending is not None:
                    epilogue(*pending)
                pending = (t, ps1)
            epilogue(*pending, split=2)

            # out stores in 2-tile chunks on the (now idle) sync ring; each
            # chunk only waits on its own slice writes, so they pipeline with
            # the epilogue tail.
            for c in range(8):
                nc.sync.dma_start(out[:, c * 2 * NB:(c + 1) * 2 * NB],
                                  o_all[:, c * 2:(c + 1) * 2, :])

    nc.compile()
    return nc


def _get_nc():
    if "nc" not in _CACHE:
        _CACHE["nc"] = _build_nc()
    return _CACHE["nc"]


def _fold_weights(conv_w: np.ndarray, w1: np.ndarray) -> np.ndarray:
    """W_eff[784, 128]: h_pre = x @ W_eff  ==  conv(x) @ w1.T  (float64 accum)."""
    w1k = w1.reshape(H, 26, 26).transpose(1, 2, 0).astype(np.float64)  # [i,j,k]
    cw = conv_w.astype(np.float64)
    W = np.zeros((28, 28, H), np.float64)
    for di in range(3):
        for dj in range(3):
            W[di:di + 26, dj:dj + 26, :] += cw[di, dj] * w1k
    return W.reshape(D, H).astype(np.float32)


def make_in_maps(x, conv_w, w1, b1, w2, b2):
    x = np.asarray(x, np.float32)
    weff = _fold_weights(np.asarray(conv_w, np.float32),
                         np.asarray(w1, np.float32))
    # wpack[p, k*128+m] = weff_pad[k*128+p, m]; wpack[p, 896+j] = w2.T padded
    wpack = np.zeros((KT, DP + KT), np.float16)
    weff_pad = np.zeros((DP, H), np.float32)
    weff_pad[:D] = weff
    wpack[:, :DP] = (weff_pad.reshape(NK, KT, H).transpose(1, 0, 2)
                     .reshape(KT, DP).astype(np.float16))
    wpack[:, DP:DP + OUT] = np.asarray(w2, np.float32).T.astype(np.float16)
    b1c = np.ascontiguousarray(np.asarray(b1, np.float32).reshape(H, 1))
    in_maps = []
    for i in range(N_CORES):
        xs = np.zeros((B_SH, DP), ml_dtypes.float8_e3m4)
        xs[:, :D] = x[i * B_SH:(i + 1) * B_SH].astype(ml_dtypes.float8_e3m4)
        # main: [t*NB+c, k*KT+p] -> [p, t, k, c]
        xtp = xs.reshape(NT, NB, NK, KT).transpose(3, 0, 2, 1)
        in_maps.append({"xtp": np.ascontiguousarray(xtp),
                        "wpack": wpack, "b1c": b1c})
    return in_maps


def kernel(x, conv_w, w1, b1, w2, b2):
    nc = _get_nc()
    in_maps = make_in_maps(x, conv_w, w1, b1, w2, b2)
    res = run_bass_kernel_spmd(nc, in_maps, list(range(N_CORES)))
    out = np.concatenate([res.results[i]["out"] for i in range(N_CORES)], axis=1)
    # [10, 65536] fp16 -> [65536, 10] fp32, + b2 (host-side fold)
    return np.ascontiguousarray(out.T.astype(np.float32)
                                + np.asarray(b2, np.float32))
